# revision 20
# baseline (speedup 1.0000x reference)
"""IVF inner-product ANN search (ApproxLinear) distributed Bass kernel for 8 TRN2 cores.

Strategy (V-sharded tensor parallel over the database):
 - Host: stable-sort vectors by IVF partition id, shard 8x contiguous, transpose shards.
 - Device per core:
   Phase 1: coarse scores qc = x @ centroids.T.
     * each core computes qc for its OWN B/8 queries over all P partitions,
       finds the 32nd-largest value per query (strided max8 + merge rounds),
       all-gathers the thresholds.
     * each core computes qc^T for its partition window and builds per-partition
       bias rows biasPT in {0, BIAS} (fp8), replicated into 64-aligned windows.
   Phase 2: per query-chunk of 128: scores tile [128q, 512v] = x @ W.T (fp32 matmul)
     + one-hot expansion matmul E (fp8) adding BIAS to non-candidate columns in PSUM.
     ACT evicts to an SBUF row; DVE does strided-chunk max8 + max_index (top-8 per
     256-chunk, decorrelated mod-stride), then 4 merge rounds -> local top-32
     (values + packed global ids via gpsimd local_scatter).
   Phase 3: AllToAll exchanges per-query candidate blocks; each core reduces the
     8x32 candidates for its own 128 queries to the global top-32 and writes
     [128, 64] = [values | ids-as-f32].
 - Host: unshard, map sorted ids back through the permutation.
"""

import numpy as np
import ml_dtypes

import concourse.bass as bass
import concourse.tile as tile
import concourse.mybir as mybir
from concourse import bacc
from concourse.mybir import dt

AF = mybir.ActivationFunctionType
ALU = mybir.AluOpType

# problem constants (full size)
B_FULL, D_FULL, V_FULL, P_FULL = 1024, 128, 200000, 8192
NCORES = 8
TOPK = 32
BIAS = -240.0          # exact in fp8 e4m3; |scores| <~ 70 so -240 dominates
EPS = 0.0              # no slack needed: mask compare is bit-exact vs threshold
KNOCK = -3.0e38        # match_replace knock-out value
VT = 512               # matmul tile width (= one PSUM bank of fp32)
SEL_CH = 256           # selection chunk length


def _cfg(B, D, V, P):
    Vl = V // NCORES
    NT = -(-Vl // VT) + 1        # tiles per core (+1 slack for window packing)
    VP = NT * VT                 # padded local V
    NT_A = NT // 2
    NT_B = NT - NT_A
    LA, LB = NT_A * VT, NT_B * VT
    nchA, nchB = LA // SEL_CH, LB // SEL_CH
    NCAND = (nchA + nchB) * 8    # local candidate count after chunk top-8
    Bq = B // NCORES
    qnch = P // 128              # threshold chunks (stride qnch, len 128)
    # fixed window schedule (identical on every core); d_eff = partitions/slot
    d_eff = (P / V) * (Vl / VP)
    m_t = [max(0, int(d_eff * VT * t) // 64) for t in range(NT)]
    n_canon = (64 * m_t[-1] + 128 + 127) // 128     # canonical 128-wide tensors
    return dict(B=B, D=D, V=V, P=P, Vl=Vl, NT=NT, VP=VP, NT_A=NT_A, NT_B=NT_B,
                LA=LA, LB=LB, nchA=nchA, nchB=nchB, NCAND=NCAND, Bq=Bq,
                qnch=qnch, m_t=m_t, n_canon=n_canon)


def build_nc(cfg, debug_taps=False):
    B, D, P = cfg["B"], cfg["D"], cfg["P"]
    VP, NT, Bq = cfg["VP"], cfg["NT"], cfg["Bq"]
    NT_A, LA, LB = cfg["NT_A"], cfg["LA"], cfg["LB"]
    nchA, nchB, NCAND = cfg["nchA"], cfg["nchB"], cfg["NCAND"]
    qnch, m_t, n_canon = cfg["qnch"], cfg["m_t"], cfg["n_canon"]
    NQC = B // 128 if B >= 128 else 1    # query chunks of (up to) 128
    QCW = min(B, 128)                    # query chunk width
    NFIN = NCORES * TOPK                 # final merge candidates (256)

    nc = bacc.Bacc(None, target_bir_lowering=False, debug=False)

    # ---- parameters ----
    f32 = dt.float32
    xT_d = nc.declare_dram_parameter("xT", [D, B], f32, isOutput=False)
    xTown_d = nc.declare_dram_parameter("xTown", [D, Bq], f32, isOutput=False)
    centT_d = nc.declare_dram_parameter("centT", [D, P], f32, isOutput=False)
    centL_d = nc.declare_dram_parameter("centTloc", [D, 128 * n_canon], f32, isOutput=False)
    Wt_d = nc.declare_dram_parameter("Wt", [D, VP], f32, isOutput=False)
    E_d = nc.declare_dram_parameter("E", [128, VP], dt.float8e4, isOutput=False)
    ident_d = nc.declare_dram_parameter("ident", [128, 128], dt.bfloat16, isOutput=False)
    cb_d = nc.declare_dram_parameter("cb", [128, NCAND], dt.uint16, isOutput=False)
    ranks1_d = nc.declare_dram_parameter("ranks1", [128, TOPK], dt.int16, isOutput=False)
    rk256u_d = nc.declare_dram_parameter("rk256u", [128, NFIN], dt.uint16, isOutput=False)
    rkoff_d = nc.declare_dram_parameter("rkoff", [128, NFIN], f32, isOutput=False)
    rkbase_d = nc.declare_dram_parameter("rkbase", [128, 1], f32, isOutput=False)
    out_d = nc.declare_dram_parameter("out", [Bq, 2 * TOPK], f32, isOutput=True)
    if debug_taps:
        dbg_t_d = nc.declare_dram_parameter("dbg_t", [1, B], f32, isOutput=True)
        dbg_c0_d = nc.declare_dram_parameter("dbg_c0", [128, B], f32, isOutput=True)
        dbg_c1_d = nc.declare_dram_parameter("dbg_c1", [128, B], f32, isOutput=True)

    with tile.TileContext(nc) as tc:
        with (
            tc.tile_pool(name="persist", bufs=1) as persist,
            tc.tile_pool(name="dram", bufs=1, space="DRAM") as dram,
        ):
            # ---- persistent SBUF ----
            xT = persist.tile([D, B], f32, tag="xT")
            E_sb = persist.tile([128, VP], dt.float8e4, tag="E_sb")
            ident = persist.tile([128, 128], dt.bfloat16, tag="ident")
            cb = persist.tile([128, NCAND], dt.uint16, tag="cb")
            ranks1 = persist.tile([128, TOPK], dt.int16, tag="ranks1")
            rk256u = persist.tile([128, NFIN], dt.uint16, tag="rk256u")
            rkoff = persist.tile([128, NFIN], f32, tag="rkoff")
            rkbase = persist.tile([128, 1], f32, tag="rkbase")
            canon = [persist.tile([128, B], dt.float8e4, tag=f"canon{i}", name=f"canon{i}") for i in range(n_canon)]
            codd = [persist.tile([128, B], dt.float8e4, tag=f"codd{i}", name=f"codd{i}") for i in range(n_canon - 1)]

            nc.sync.dma_start(xT[:], xT_d[:])
            nc.sync.dma_start(E_sb[:], E_d[:])
            nc.sync.dma_start(ident[:], ident_d[:])
            nc.sync.dma_start(cb[:], cb_d[:])
            nc.sync.dma_start(ranks1[:], ranks1_d[:])
            nc.sync.dma_start(rk256u[:], rk256u_d[:])
            nc.sync.dma_start(rkoff[:], rkoff_d[:])
            nc.sync.dma_start(rkbase[:], rkbase_d[:])

            # collective bounces
            t_in = dram.tile([Bq, 1], f32, tag="t_in")
            t_out = dram.tile([1, B], f32, tag="t_out", addr_space="Shared")
            a2a_in = dram.tile([B, 2 * TOPK], f32, tag="a2a_in")
            a2a_out = dram.tile([B, 2 * TOPK], f32, tag="a2a_out")

            # ================= phase 1: thresholds + bias =================
            with (
                tc.tile_pool(name="p1", bufs=1) as p1,
                tc.tile_pool(name="p1psum", bufs=2, space=bass.MemorySpace.PSUM) as p1psum,
            ):
                xTown = p1.tile([D, Bq], f32, tag="xTown")
                centT = p1.tile([D, P], f32, tag="centT")
                centL = p1.tile([D, 128 * n_canon], f32, tag="centL")
                qc = p1.tile([Bq, P], f32, tag="qc")
                nc.sync.dma_start(xTown[:], xTown_d[:])
                nc.sync.dma_start(centT[:], centT_d[:])
                nc.sync.dma_start(centL[:], centL_d[:])

                # qc for own queries
                for i in range(P // VT):
                    ps = p1psum.tile([Bq, VT], f32)
                    nc.tensor.matmul(ps[:], xTown[:], centT[:, i * VT:(i + 1) * VT],
                                     start=True, stop=True)
                    nc.scalar.activation(qc[:, i * VT:(i + 1) * VT], ps[:], AF.Copy)

                # threshold = 32nd largest of each qc row (strided chunk top-8)
                m8t = p1.tile([Bq, qnch * 8], f32, tag="m8t")
                for j in range(qnch):
                    nc.vector.max(m8t[:, j * 8:(j + 1) * 8], qc[:, j::qnch])
                t8 = p1.tile([Bq, TOPK], f32, tag="t8")
                for r in range(TOPK // 8):
                    nc.vector.max(t8[:, r * 8:(r + 1) * 8], m8t[:])
                    if r < TOPK // 8 - 1:
                        nc.vector.match_replace(m8t[:], t8[:, r * 8:(r + 1) * 8],
                                                m8t[:], KNOCK)
                nc.sync.dma_start(t_in[:], t8[:, TOPK - 1:TOPK])

                nc.gpsimd.collective_compute(
                    "AllGather", ALU.bypass,
                    replica_groups=[list(range(NCORES))],
                    ins=[t_in.opt()], outs=[t_out.opt()],
                )

                # per-partition thresholds per q-chunk: tq[p, i] = t[i*128 + p]
                tq = p1.tile([128, NQC], f32, tag="tq")
                nc.sync.dma_start(
                    tq[:], t_out[:].rearrange("o (i p) -> p (o i)", p=QCW))

                # window scores in q-orientation, mask = (qc >= t) as exact 0/1,
                # then TensorE-transpose to partition-major and scale to bias.
                WCOL = 128 * n_canon
                for qi in range(NQC):
                    msk = p1.tile([128, WCOL], dt.bfloat16, tag="msk")
                    for wh in range(-(-WCOL // VT)):
                        w = min(VT, WCOL - wh * VT)
                        ps = p1psum.tile([128, VT], f32, tag="mps")
                        nc.tensor.matmul(ps[:, :w], xT[:, qi * QCW:(qi + 1) * QCW],
                                         centL[:, wh * VT:wh * VT + w],
                                         start=True, stop=True)
                        nc.vector.tensor_scalar(msk[:, wh * VT:wh * VT + w],
                                                ps[:, :w], tq[:, qi:qi + 1],
                                                None, ALU.is_ge)
                    for pc in range(n_canon):
                        tp = p1psum.tile([128, 128], dt.bfloat16, tag="tps")
                        nc.tensor.transpose(tp[:], msk[:, pc * 128:(pc + 1) * 128],
                                            ident[:])
                        nc.vector.tensor_scalar(
                            canon[pc][:, qi * QCW:(qi + 1) * QCW], tp[:],
                            float(-BIAS), float(BIAS), ALU.mult, ALU.add)
                if debug_taps:
                    nc.sync.dma_start(dbg_t_d[:], t_out[:])
                    dbgc = p1.tile([128, B], f32, tag="dbgc")
                    nc.vector.tensor_scalar(dbgc[:], canon[0][:], 0.0, None, ALU.add)
                    nc.sync.dma_start(dbg_c0_d[:], dbgc[:])
                    dbgc1 = p1.tile([128, B], f32, tag="dbgc1")
                    nc.vector.tensor_scalar(dbgc1[:], canon[1][:], 0.0, None, ALU.add)
                    nc.sync.dma_start(dbg_c1_d[:], dbgc1[:])
                # odd (64-shifted) windows
                for i in range(n_canon - 1):
                    nc.sync.dma_start(codd[i][0:64, :], canon[i][64:128, :])
                    nc.sync.dma_start(codd[i][64:128, :], canon[i + 1][0:64, :])

            # window m -> lhsT tensor
            def biasW(m):
                return canon[m // 2] if m % 2 == 0 else codd[(m - 1) // 2]

            # ================= phase 2: main matmul + local top-k =================
            with (
                tc.tile_pool(name="scores", bufs=2) as scores_pool,
                tc.tile_pool(name="wtiles", bufs=4) as wt_pool,
                tc.tile_pool(name="psum", bufs=6, space=bass.MemorySpace.PSUM) as psum_pool,
                tc.tile_pool(name="sel", bufs=2) as sel,
            ):
                for qi in range(NQC):
                    qsl = slice(qi * QCW, (qi + 1) * QCW)
                    m8 = sel.tile([QCW, NCAND], f32, tag="m8")
                    i8 = sel.tile([QCW, NCAND], dt.uint16, tag="i8")
                    for half in range(2):
                        t0 = 0 if half == 0 else NT_A
                        t1 = NT_A if half == 0 else NT
                        LH = LA if half == 0 else LB
                        nch = nchA if half == 0 else nchB
                        choff = 0 if half == 0 else nchA
                        sc = scores_pool.tile([QCW, LH], f32)
                        for t in range(t0, t1):
                            wt = wt_pool.tile([D, VT], f32)
                            nc.sync.dma_start(wt[:], Wt_d[:, t * VT:(t + 1) * VT])
                            ps = psum_pool.tile([QCW, VT], f32)
                            nc.tensor.matmul(ps[:], xT[:, qsl], wt[:],
                                             start=True, stop=False)
                            nc.tensor.matmul(ps[:], biasW(m_t[t])[:, qsl],
                                             E_sb[:, t * VT:(t + 1) * VT],
                                             start=False, stop=True)
                            off = (t - t0) * VT
                            nc.scalar.activation(sc[:, off:off + VT], ps[:], AF.Copy)
                        for j in range(nch):
                            o8 = (choff + j) * 8
                            nc.vector.max(m8[:, o8:o8 + 8], sc[:, j::nch])
                            nc.vector.max_index(i8[:, o8:o8 + 8], m8[:, o8:o8 + 8],
                                                sc[:, j::nch])

                    # local ids: vg = cb + stride * i8 (u16 exact)
                    vg = sel.tile([QCW, NCAND], dt.uint16, tag="vg")
                    nc.vector.tensor_scalar(vg[:, :nchA * 8], i8[:, :nchA * 8],
                                            nchA, None, ALU.mult)
                    nc.vector.tensor_scalar(vg[:, nchA * 8:], i8[:, nchA * 8:],
                                            nchB, None, ALU.mult)
                    nc.vector.tensor_tensor(vg[:], vg[:], cb[:QCW, :], ALU.add)

                    # merge rounds: top-32 of NCAND (+ positions)
                    wv = sel.tile([QCW, TOPK], f32, tag="wv")
                    wp = sel.tile([QCW, TOPK], dt.uint16, tag="wp")
                    for r in range(TOPK // 8):
                        s8 = slice(r * 8, r * 8 + 8)
                        nc.vector.max(wv[:, s8], m8[:])
                        nc.vector.max_index(wp[:, s8], wv[:, s8], m8[:])
                        if r < TOPK // 8 - 1:
                            nc.vector.match_replace(m8[:], wv[:, s8], m8[:], KNOCK)

                    # gather vg at winner positions via local_scatter x2
                    slot = sel.tile([QCW, NCAND], dt.uint16, tag="slot")
                    nc.gpsimd.local_scatter(slot[:], ranks1[:QCW, :],
                                            wp[:].bitcast(dt.int16),
                                            channels=QCW, num_elems=NCAND,
                                            num_idxs=TOPK)
                    slotm1 = sel.tile([QCW, NCAND], dt.int16, tag="slotm1")
                    nc.vector.tensor_scalar(slotm1[:], slot[:].bitcast(dt.int16),
                                            1, None, ALU.subtract)
                    vwin = sel.tile([QCW, TOPK], dt.uint16, tag="vwin")
                    nc.gpsimd.local_scatter(vwin[:], vg[:], slotm1[:],
                                            channels=QCW, num_elems=TOPK,
                                            num_idxs=NCAND)
                    # candidate block [QCW, 64] = [values | rkbase + vwin]
                    blk = sel.tile([QCW, 2 * TOPK], f32, tag="blk")
                    nc.vector.tensor_scalar(blk[:, 0:TOPK], wv[:], 0.0, None, ALU.add)
                    nc.vector.tensor_scalar(blk[:, TOPK:], vwin[:],
                                            rkbase[:QCW, :], None, ALU.add)
                    nc.sync.dma_start(a2a_in[qsl, :], blk[:])

            # ================= phase 3: exchange + final top-k =================
            with (
                tc.tile_pool(name="fin", bufs=1) as fin,
            ):
                nc.gpsimd.collective_compute(
                    "AllToAll", ALU.bypass,
                    replica_groups=[list(range(NCORES))],
                    ins=[a2a_in.opt()], outs=[a2a_out.opt()],
                )
                vals = fin.tile([Bq, NFIN], f32, tag="vals")
                ids = fin.tile([Bq, NFIN], f32, tag="ids")
                a2a_v = a2a_out[:].rearrange("(r q) d -> q r d", r=NCORES)
                nc.sync.dma_start(vals[:].rearrange("q (r k) -> q r k", r=NCORES),
                                  a2a_v[:, :, 0:TOPK])
                nc.sync.dma_start(ids[:].rearrange("q (r k) -> q r k", r=NCORES),
                                  a2a_v[:, :, TOPK:2 * TOPK])

                osb = fin.tile([Bq, 2 * TOPK], f32, tag="osb")
                wp2 = fin.tile([Bq, TOPK], dt.uint16, tag="wp2")
                for r in range(TOPK // 8):
                    s8 = slice(r * 8, r * 8 + 8)
                    nc.vector.max(osb[:, s8], vals[:])
                    nc.vector.max_index(wp2[:, s8], osb[:, s8], vals[:])
                    if r < TOPK // 8 - 1:
                        nc.vector.match_replace(vals[:], osb[:, s8], vals[:], KNOCK)

                # id planes -> u16 local id + u16 rank
                idl = fin.tile([Bq, NFIN], f32, tag="idl")
                nc.vector.tensor_tensor(idl[:], ids[:], rkoff[:Bq, :], ALU.subtract)
                idl16 = fin.tile([Bq, NFIN], dt.uint16, tag="idl16")
                nc.vector.tensor_scalar(idl16[:], idl[:], 0.0, None, ALU.add)

                slot2 = fin.tile([Bq, NFIN], dt.uint16, tag="slot2")
                nc.gpsimd.local_scatter(slot2[:], ranks1[:Bq, :],
                                        wp2[:].bitcast(dt.int16),
                                        channels=Bq, num_elems=NFIN, num_idxs=TOPK)
                slot2m1 = fin.tile([Bq, NFIN], dt.int16, tag="slot2m1")
                nc.vector.tensor_scalar(slot2m1[:], slot2[:].bitcast(dt.int16),
                                        1, None, ALU.subtract)
                idwin = fin.tile([Bq, TOPK], dt.uint16, tag="idwin")
                nc.gpsimd.local_scatter(idwin[:], idl16[:], slot2m1[:],
                                        channels=Bq, num_elems=TOPK, num_idxs=NFIN)
                rkwin = fin.tile([Bq, TOPK], dt.uint16, tag="rkwin")
                nc.gpsimd.local_scatter(rkwin[:], rk256u[:Bq, :], slot2m1[:],
                                        channels=Bq, num_elems=TOPK, num_idxs=NFIN)

                # I = idwin + VP * rkwin (as f32, exact)
                idf = fin.tile([Bq, TOPK], f32, tag="idf")
                rkf = fin.tile([Bq, TOPK], f32, tag="rkf")
                nc.vector.tensor_scalar(idf[:], idwin[:], 0.0, None, ALU.add)
                nc.vector.tensor_scalar(rkf[:], rkwin[:], float(VP), None, ALU.mult)
                nc.vector.tensor_tensor(osb[:, TOPK:], idf[:], rkf[:], ALU.add)

                nc.sync.dma_start(out_d[:], osb[:])

    nc.compile()
    return nc


def pack_core(rel, cfg):
    """Greedy-pack sorted vectors (rel = partition offsets, non-decreasing) into
    VT-slot tiles so tile t's partitions fit window [64*m_t, 64*m_t+128)."""
    VT_, NT, VP, Vl = VT, cfg["NT"], cfg["VP"], cfg["Vl"]
    m_t = cfg["m_t"]
    slots = np.full(VP, -1, np.int64)
    v = 0
    for t in range(NT):
        lo, hi = 64 * m_t[t], 64 * m_t[t] + 128
        cnt = 0
        while cnt < VT_ and v < Vl and rel[v] < hi:
            assert rel[v] >= lo, f"tile {t}: vector below window ({rel[v]} < {lo})"
            slots[t * VT_ + cnt] = v
            v += 1
            cnt += 1
    assert v == Vl, f"{Vl - v} vectors unplaced (window schedule too tight)"
    return slots


def host_prepare(x, W, centroids, assign, cfg):
    """Build per-core in_maps + the permutation for unsharding."""
    B, D, P, V = cfg["B"], cfg["D"], cfg["P"], cfg["V"]
    Vl, VP, NT, Bq = cfg["Vl"], cfg["VP"], cfg["NT"], cfg["Bq"]
    NCAND, nchA, nchB, NT_A = cfg["NCAND"], cfg["nchA"], cfg["nchB"], cfg["NT_A"]
    m_t, n_canon = cfg["m_t"], cfg["n_canon"]
    NFIN = NCORES * TOPK

    x = np.ascontiguousarray(x, np.float32)
    W = np.ascontiguousarray(W, np.float32)
    centroids = np.ascontiguousarray(centroids, np.float32)
    assign = np.asarray(assign)

    perm = np.argsort(assign, kind="stable")
    Ws = W[perm]
    As = assign[perm].astype(np.int64)

    xT = np.ascontiguousarray(x.T)                      # [D, B]
    centT = np.ascontiguousarray(centroids.T)           # [D, P]

    # shared consts
    ident = np.eye(128).astype(ml_dtypes.bfloat16)
    cb = np.zeros(NCAND, np.uint16)
    for j in range(nchA):
        cb[j * 8:(j + 1) * 8] = j
    for j in range(nchB):
        o = (nchA + j) * 8
        cb[o:o + 8] = NT_A * VT + j
    cb = np.broadcast_to(cb, (128, NCAND)).copy()
    ranks1 = np.broadcast_to(np.arange(1, TOPK + 1, dtype=np.int16), (128, TOPK)).copy()
    rk256u = np.broadcast_to(
        (np.arange(NFIN) // TOPK).astype(np.uint16), (128, NFIN)).copy()
    rkoff = np.broadcast_to(
        ((np.arange(NFIN) // TOPK) * VP).astype(np.float32), (128, NFIN)).copy()

    in_maps = []
    slot_orig = np.full((NCORES, VP), -1, np.int64)
    for c in range(NCORES):
        lo = c * Vl
        Wc = Ws[lo:lo + Vl]
        Ac = As[lo:lo + Vl]
        wbase = int(Ac[0])
        rel = (Ac - wbase).astype(np.int64)
        slots = pack_core(rel, cfg)          # [VP] -> local sorted idx or -1
        placed = slots >= 0
        Wt = np.zeros((D, VP), np.float32)
        Wt[:, placed] = Wc.T[:, slots[placed]]
        E = np.zeros((128, VP), ml_dtypes.float8_e4m3)
        svec = np.where(placed)[0]
        tiles = svec // VT
        rows = rel[slots[svec]] - 64 * np.asarray(m_t, np.int64)[tiles]
        assert rows.min() >= 0 and rows.max() < 128
        E[rows, svec] = ml_dtypes.float8_e4m3(1.0)
        slot_orig[c, placed] = perm[lo + slots[placed]]
        centL = np.zeros((D, 128 * n_canon), np.float32)
        hi = min(P, wbase + 128 * n_canon)
        centL[:, :hi - wbase] = centT[:, wbase:hi]
        rkbase = np.full((128, 1), c * VP, np.float32)

        in_maps.append({
            "xT": xT, "xTown": np.ascontiguousarray(xT[:, c * Bq:(c + 1) * Bq]),
            "centT": centT, "centTloc": centL, "Wt": Wt, "E": E,
            "ident": ident, "cb": cb, "ranks1": ranks1, "rk256u": rk256u,
            "rkoff": rkoff, "rkbase": rkbase,
        })
    return in_maps, slot_orig


def host_assemble(outs, slot_orig, cfg):
    """outs: list of per-core 'out' arrays [Bq, 64] -> (I [B,32] int32, D [B,32] f32)."""
    B, VP, Bq = cfg["B"], cfg["VP"], cfg["Bq"]
    Dv = np.zeros((B, TOPK), np.float32)
    I = np.zeros((B, TOPK), np.int32)
    for c in range(NCORES):
        o = outs[c]
        Dv[c * Bq:(c + 1) * Bq] = o[:, :TOPK]
        gid = o[:, TOPK:].astype(np.int64)
        orig = slot_orig[gid // VP, gid % VP]
        assert orig.min() >= 0, "pad slot leaked into final top-k"
        I[c * Bq:(c + 1) * Bq] = orig.astype(np.int32)
    return I, Dv


_NC_CACHE = {}


def kernel(x, W, centroids, assign):
    from concourse import bass_utils
    cfg = _cfg(B_FULL, D_FULL, V_FULL, P_FULL)
    key = (cfg["B"], cfg["V"])
    if key not in _NC_CACHE:
        _NC_CACHE[key] = build_nc(cfg)
    nc = _NC_CACHE[key]
    in_maps, slot_orig = host_prepare(x, W, centroids, assign, cfg)
    res = bass_utils.run_bass_kernel_spmd(nc, in_maps, core_ids=list(range(NCORES)))
    outs = [res.results[c]["out"] for c in range(NCORES)]
    return host_assemble(outs, slot_orig, cfg)
